# revision 42
# baseline (speedup 1.0000x reference)
"""Causal self-attention (dense transformer block) on 8 Trainium2 NeuronCores.

Problem (hardcoded): B=2, S=2048, HID=2048, NH=16, HS=128, ROT=32 (partial
rotary), causal additive mask, f32 I/O.

Sharding: core c = b*4 + g handles batch b and head-group g (4 heads).
 - Phase A (projections): qkT = (x @ Wqk_shard)^T computed from host-provided
   xT (x[b] transposed, pre-cast bf16) so the contraction dim lands on
   partitions.  Q,K are produced transposed ([d, s]); V natural ([s, d]).
   m-block 0 runs a kb-OUTER sweep across all 8 feature chains (8 psum
   banks) with kb-major weight streaming, so the PE chases the incoming
   xT0/wqk DMA stream instead of stalling ~13us on the first chain.
 - RoPE in-place on the first 32 dims of each head of Q,K; rotate_half is a
   32x32 sign-permutation matmul.  All rope elementwise ops on DVE: the
   GpSimd engine stream carries the collective doorbells and must stay clear.
 - Phase B (attention): S^T tiles [128 k, 512 q] on PE, exp on ACT (no max
   subtraction; scores are O(5)), causality structural: upper blocks skipped,
   diagonal-block masking is a -1e9 pattern accumulated INTO the score psum
   by an identity-stationary matmul (128 PE cycles; exp then yields exact
   zeros, nothing on DVE).  AV accumulated on PE.  Softmax denominators:
   exp'd tiles accumulated elementwise on DVE only (bf16 SBUF adds run 2x;
   Pool would contend on the shared SBUF port), first exp written straight
   into the accumulator, one ones-matmul per head for the cross-partition
   sum.
 - Per-chunk AllGather (groups of 4 = same batch) after every 2 heads.
   Bounce writes + doorbells ride the gpsimd/SWDGE queue (on the sync queue
   they'd sit behind MBs of queued x/weight bulk and delay gathers ~40us);
   all bulk (x, weights, ga reads, out stores) rides the sync/HWDGE queue.
 - Dense (attn @ Wdense) is split into ~0.5us PE units: c(0)/c(1) fill the
   PE slack inside the ACT-bound attention phases b(3)/b(2); the tail runs
   the leftover half + c(3) + c(2), with the last 8 fill units (on the free
   'av' psum ring) inserted between c(2)'s two chunks to cover the final
   gather's in-flight window (no HAM cold restart).
 - Head h's softmax finalize (sum matmul / recip / bounce DMA / gather) is
   emitted 2 score-blocks into head h+1 so its DVE chain never head-of-line
   blocks the PE.
 - Host reassembles the 8 [512, 2048] transposed bf16 output slices.
"""

import numpy as np
import ml_dtypes

import concourse.bass as bass
import concourse.bacc as bacc
import concourse.mybir as mybir
import concourse.tile as tile
from concourse import bass_utils

B, S, HID = 2, 2048, 2048
NH = 16
HS = 128
ROT = 32
BASE = 10000.0
G = 4            # head-groups (4 heads each)
HPG = NH // G    # heads per core = 4
NCORES = 8

MB = 4           # m (seq) blocks of 512
KB = 16          # contraction blocks of 128
NBQK = 2 * HPG   # qk feature blocks of 128 (q,k interleaved per head)
F32 = mybir.dt.float32
BF16 = mybir.dt.bfloat16
SCALE = 1.0 / float(np.sqrt(np.float32(HS)))

_CACHE = {}


def _build_nc():
    nc = bacc.Bacc(
        "TRN2", target_bir_lowering=False, debug=False, num_devices=NCORES
    )

    io = {
        "xT": nc.dram_tensor("xT", [HID, S], BF16, kind="ExternalInput"),
        # first m-block of xT, contiguous [HID, 512]: per-tile dram ranges
        # are sequential so the head DMAs aggregate into large packets
        "xT0": nc.dram_tensor("xT0", [HID, 512], BF16, kind="ExternalInput"),
        # kb-major: [kb, p, nb*128+n] -- one contiguous 256KB tile per kb
        "wqk": nc.dram_tensor("wqk", [KB, 128, NBQK * 128], BF16, kind="ExternalInput"),
        # contiguous duplicate of wqk[0][:, 0:128] so the very first matmul's
        # stationary is one 32KB sequential DMA on the early-starting queue
        "w0at": nc.dram_tensor("w0at", [128, 128], BF16, kind="ExternalInput"),
        "wv": nc.dram_tensor("wv", [KB, 128, 512], BF16, kind="ExternalInput"),
        "wd": nc.dram_tensor("wd", [KB, 128, 512], BF16, kind="ExternalInput"),
        "bqk": nc.dram_tensor("bqk", [NBQK * 128], F32, kind="ExternalInput"),
        "bv": nc.dram_tensor("bv", [HPG * 128], F32, kind="ExternalInput"),
        "bd": nc.dram_tensor("bd", [512], F32, kind="ExternalInput"),
        "cosb": nc.dram_tensor("cosb", [ROT, S], BF16, kind="ExternalInput"),
        "sinb": nc.dram_tensor("sinb", [ROT, S], BF16, kind="ExternalInput"),
        "rt": nc.dram_tensor("rt", [ROT, ROT], BF16, kind="ExternalInput"),
        "idm": nc.dram_tensor("idm", [128, 128], BF16, kind="ExternalInput"),
        "mneg": nc.dram_tensor("mneg", [128, 128], BF16, kind="ExternalInput"),
        "outT": nc.dram_tensor("outT", [512, S], BF16, kind="ExternalOutput"),
    }

    with tile.TileContext(nc) as tc:
        _kernel_body(tc, io)

    nc.compile()
    return nc


def _kernel_body(tc, io):
    nc = tc.nc
    from contextlib import ExitStack

    with ExitStack() as ctx:
        singles = ctx.enter_context(tc.tile_pool(name="singles", bufs=1))
        wpool = ctx.enter_context(tc.tile_pool(name="wpool", bufs=1))
        work = ctx.enter_context(tc.tile_pool(name="work", bufs=2))
        psum = ctx.enter_context(tc.tile_pool(name="psum", bufs=2, space="PSUM"))
        dram = ctx.enter_context(tc.tile_pool(name="dram", bufs=1, space="DRAM"))

        # ---- first-matmul critical path on the SWDGE queue: it spins up
        # ~6us before the HWDGE sync queue, so the first chain's stationary
        # and first moving tiles go here, ahead of the small constants ----
        w0a = wpool.tile([128, 128], BF16, tag="wqk0a", name="w0a")
        nc.gpsimd.dma_start(out=w0a, in_=io["w0at"][:, :])
        xbf0 = []
        for kb in range(4):
            xb = work.tile([128, 512], BF16, tag="xbf", bufs=28, name="xb")
            nc.gpsimd.dma_start(
                out=xb, in_=io["xT0"][128 * kb : 128 * (kb + 1), :]
            )
            xbf0.append(xb)
        # tiny constants next: the nb=0 bias-add gates the acc-psum ring
        bqk_sb = singles.tile([128, NBQK], F32, tag="bqk_sb")
        nc.gpsimd.dma_start(out=bqk_sb, in_=io["bqk"].rearrange("(n p) -> p n", p=128))
        bd_sb = singles.tile([128, 4], F32, tag="bd_sb")
        nc.gpsimd.dma_start(out=bd_sb, in_=io["bd"].rearrange("(n p) -> p n", p=128))
        bv_row = singles.tile([1, HPG * 128], F32, tag="bv_row")
        nc.gpsimd.dma_start(out=bv_row, in_=io["bv"][None, :])
        bvB = singles.tile([128, HPG * 128], F32, tag="bvB")
        nc.gpsimd.partition_broadcast(bvB, bv_row)
        rt_sb = singles.tile([ROT, ROT], BF16, tag="rt_sb")
        nc.gpsimd.dma_start(out=rt_sb, in_=io["rt"][:, :])
        cos_sb = singles.tile([ROT, S], BF16, tag="cos_sb")
        nc.gpsimd.dma_start(out=cos_sb, in_=io["cosb"][:, :])
        sin_sb = singles.tile([ROT, S], BF16, tag="sin_sb")
        nc.gpsimd.dma_start(out=sin_sb, in_=io["sinb"][:, :])
        ones_sb = singles.tile([128, 128], BF16, tag="ones_sb")
        nc.vector.memset(ones_sb, 1.0)

        # ---- sync (HWDGE) queue: wqk is kb-major ([kb, p, nb*128+n]) and
        # streamed interleaved with the xT0 tiles in the kb-outer sweep's
        # consumption order, so the first projection phase chases the DMA
        # stream instead of stalling on the full xT0 transfer
        wkb_sb = [None] * KB
        wkb0r = wpool.tile([128, (NBQK - 1) * 128], BF16, tag="wqk0b", name="w0b")
        nc.sync.dma_start(out=wkb0r, in_=io["wqk"][0][:, 128:])

        def wqk_slice(nb, kb):
            if kb == 0:
                return w0a if nb == 0 else wkb0r[:, 128 * (nb - 1) : 128 * nb]
            return wkb_sb[kb][:, 128 * nb : 128 * (nb + 1)]

        def load_wkb(kb, eng):
            w_t = wpool.tile(
                [128, NBQK * 128], BF16, tag=f"wkb{kb}", name=f"wkb{kb}"
            )
            eng.dma_start(out=w_t, in_=io["wqk"][kb])
            wkb_sb[kb] = w_t

        for kb in range(4, KB):
            xb = work.tile([128, 512], BF16, tag="xbf", bufs=28, name="xb")
            nc.sync.dma_start(
                out=xb, in_=io["xT0"][128 * kb : 128 * (kb + 1), :]
            )
            xbf0.append(xb)
            if kb - 3 < 8:
                load_wkb(kb - 3, nc.sync)
        # wkb 8-11 ride the otherwise-idle gpsimd queue; the rest close out
        # the sync stream just ahead of the sweep's tail
        for kb in range(8, 12):
            load_wkb(kb, nc.gpsimd)
        for kb in range(12, KB):
            load_wkb(kb, nc.sync)

        id_sb = singles.tile([128, 128], BF16, tag="id_sb")
        nc.sync.dma_start(out=id_sb, in_=io["idm"][:, :])
        mneg_sb = singles.tile([128, 128], BF16, tag="mneg_sb")
        nc.sync.dma_start(out=mneg_sb, in_=io["mneg"][:, :])

        wv_sb = []
        wd_sb = []

        def load_wv():
            for kb in range(KB):
                wv_t = wpool.tile([128, 512], BF16, tag=f"wv{kb}", name=f"wv{kb}")
                nc.sync.dma_start(out=wv_t, in_=io["wv"][kb])
                wv_sb.append(wv_t)

        def load_wd():
            for kb in range(KB):
                wd_t = wpool.tile([128, 512], BF16, tag=f"wd{kb}", name=f"wd{kb}")
                nc.sync.dma_start(out=wd_t, in_=io["wd"][kb])
                wd_sb.append(wd_t)

        # ---- DRAM bounce buffers, one per (q-block, chunk) ----
        # chunked AllGathers start as soon as their heads finish.  j=2 is the
        # last attention phase: its final two chunks are single heads so the
        # last collective (the one the tail dense phase waits on) is small
        # and ends as early as possible.
        CHUNKS = {
            0: [[0, 1], [2, 3]],
            1: [[0, 1], [2, 3]],
            3: [[0, 1], [2, 3]],
            2: [[2, 3], [0, 1]],
        }

        def chunk_loc(j, h):
            for ci, ch in enumerate(CHUNKS[j]):
                if h in ch:
                    return ci, ch.index(h)

        bounce = {}
        gath = {}
        for j in range(MB):
            for ci, ch in enumerate(CHUNKS[j]):
                bounce[(j, ci)] = dram.tile(
                    [len(ch) * 128, 512], BF16,
                    tag=f"bounce{j}_{ci}", name=f"bounce{j}_{ci}",
                )
                gath[(j, ci)] = dram.tile(
                    [G * len(ch) * 128, 512], BF16,
                    tag=f"gath{j}_{ci}", name=f"gath{j}_{ci}",
                )

        # ---- persistent qkT / V tiles ----
        qkT = {}
        for nb in range(NBQK):
            for mb in range(MB):
                qkT[(nb, mb)] = wpool.tile(
                    [128, 512], BF16, tag=f"qkT_{nb}_{mb}", name=f"qkT_{nb}_{mb}"
                )
        v_sb = []
        for km in range(KB):
            v_sb.append(
                wpool.tile([128, HPG * 128], BF16, tag=f"v_{km}", name=f"v_{km}")
            )

        # deferred per-head softmax finalizers (sum matmul / recip / bounce /
        # gather): emitted a couple of PE ops into the NEXT head so the
        # Pool+DVE denominator chain never stalls the PE
        pending_fin = []

        def flush_fin():
            while pending_fin:
                pending_fin.pop(0)()

        # ============ phase emitters ============

        def phase_a(mb):
            """projections for m-block mb + RoPE"""
            if mb == 0:
                xbf = xbf0
            else:
                xbf = []
                for kb in range(KB):
                    xb = work.tile([128, 512], BF16, tag="xbf", bufs=28, name="xb")
                    nc.sync.dma_start(
                        out=xb,
                        in_=io["xT"][
                            128 * kb : 128 * (kb + 1), 512 * mb : 512 * (mb + 1)
                        ],
                    )
                    xbf.append(xb)

            csl = cos_sb[:, 512 * mb : 512 * (mb + 1)]
            ssl = sin_sb[:, 512 * mb : 512 * (mb + 1)]

            def emit_rope(nb):
                # RoPE per feature block (a' = a*cos + rot*sin, rotate_half
                # as a 32x32 sign-permutation matmul). Emitted one block
                # late so the PE never waits on the bias-add -> rope chain.
                qk = nb % 2
                a = qkT[(nb, mb)]
                ps_r = psum.tile([ROT, 512], F32, tag="score", bufs=3, name="ps_r")
                nc.tensor.matmul(ps_r, rt_sb, a[0:ROT, :], start=True, stop=True)
                # all three elementwise ops on DVE: the GpSimd engine stream
                # must stay clear -- it carries the collective enqueues, and a
                # rope op stuck behind a DVE dependency would stall them
                tq = work.tile([ROT, 512], BF16, tag=f"ropeq{qk}", bufs=1)
                nc.vector.tensor_mul(out=tq, in0=a[0:ROT, :], in1=csl)
                ts = work.tile([ROT, 512], BF16, tag=f"ropes{qk}", bufs=1)
                nc.vector.tensor_mul(out=ts, in0=ps_r, in1=ssl)
                nc.vector.tensor_add(out=a[0:ROT, :], in0=tq, in1=ts)

            if mb == 0:
                # kb-outer sweep across all 8 feature chains (one psum bank
                # each, using the full 8-bank budget -- nothing else is in
                # flight yet): each x tile is consumed 8x the moment it lands,
                # so the PE tracks the xT0/wqk DMA stream instead of stalling
                # ~13us for the full transfer.  Bias-adds all emitted before
                # the ropes so the 'score'-ring ps_r allocs never deadlock
                # against the still-open chain banks.
                ptags = [
                    ("acc", 2), ("acc", 2), ("score", 3), ("score", 3),
                    ("score", 3), ("av", 2), ("av", 2), ("sum", 1),
                ]
                pss = [
                    psum.tile([128, 512], F32, tag=t, bufs=b, name=f"ps_sw{nb}")
                    for nb, (t, b) in enumerate(ptags)
                ]
                for kb in range(KB):
                    for nb in range(NBQK):
                        nc.tensor.matmul(
                            pss[nb],
                            wqk_slice(nb, kb),
                            xbf[kb],
                            start=(kb == 0),
                            stop=(kb == KB - 1),
                        )
                for nb in range(NBQK):
                    nc.vector.tensor_scalar_add(
                        out=qkT[(nb, mb)], in0=pss[nb],
                        scalar1=bqk_sb[:, nb : nb + 1],
                    )
                for nb in range(NBQK):
                    emit_rope(nb)
            else:
                for nb in range(NBQK):
                    ps = psum.tile([128, 512], F32, tag="acc", bufs=2, name="ps_qk")
                    for kb in range(KB):
                        nc.tensor.matmul(
                            ps,
                            wqk_slice(nb, kb),
                            xbf[kb],
                            start=(kb == 0),
                            stop=(kb == KB - 1),
                        )
                    if nb == 1:
                        flush_fin()
                    nc.vector.tensor_scalar_add(
                        out=qkT[(nb, mb)], in0=ps, scalar1=bqk_sb[:, nb : nb + 1]
                    )
                    if nb > 0:
                        emit_rope(nb - 1)
                emit_rope(NBQK - 1)

            if mb == 0:
                load_wv()
            for msub in range(4):
                km = 4 * mb + msub
                ps = psum.tile([128, HPG * 128], F32, tag="acc", bufs=2, name="ps_v")
                for kb in range(KB):
                    nc.tensor.matmul(
                        ps,
                        xbf[kb][:, 128 * msub : 128 * (msub + 1)],
                        wv_sb[kb],
                        start=(kb == 0),
                        stop=(kb == KB - 1),
                    )
                nc.vector.tensor_add(out=v_sb[km], in0=ps, in1=bvB)

        def phase_b(j, fill=0):
            """attention for q-block j (all heads); per-head finalize is
            deferred into the next head (or next phase) via pending_fin.
            `fill` dense c-units are emitted per head: the attention heads
            are ACT(exp)-bound, so the PE slack absorbs the dense matmuls"""
            nkm = 4 * j + 4
            # heads visit in chunk order so each chunk's gather fires as soon
            # as its last head finishes
            order = tuple(h for ch in CHUNKS[j] for h in ch)
            for pos, h in enumerate(order):
                qt = qkT[(2 * h, j)]

                def mk_u(i, qt=qt, out=None):
                    # diagonal blocks only need q-columns >= 128*(i-4j): compute
                    # the triangular remainder; causal masking of the first 128
                    # columns is a -1e9 accumulate on the PE (identity
                    # stationary x pattern moving, 128 cycles) so exp yields
                    # exact zeros and the DVE never touches it
                    qoff = max(0, 128 * (i - 4 * j))
                    width = 512 - qoff
                    kt = qkT[(2 * h + 1, i // 4)]
                    diag = i >= 4 * j
                    ps_s = psum.tile([128, 512], F32, tag="score", bufs=3, name="ps_s")
                    nc.tensor.matmul(
                        ps_s[:, 0:width],
                        kt[:, 128 * (i % 4) : 128 * (i % 4 + 1)],
                        qt[:, qoff:512],
                        start=True,
                        stop=not diag,
                    )
                    if diag:
                        nc.tensor.matmul(
                            ps_s[:, 0:128],
                            id_sb,
                            mneg_sb,
                            start=False,
                            stop=True,
                        )
                    u = out
                    if u is None:
                        u = work.tile([128, 512], BF16, tag="u", bufs=8, name="u")
                    nc.scalar.activation(
                        out=u[:, 0:width], in_=ps_s[:, 0:width],
                        func=mybir.ActivationFunctionType.Exp, scale=SCALE,
                    )
                    return u, qoff, width

                ps_av = psum.tile([128, 512], F32, tag="av", bufs=2, name="ps_av")
                # accumulate exp'd tiles elementwise (the PE is the global
                # bottleneck; a single ones-matmul per head does the
                # cross-partition sum).  All adds on DVE: bf16 SBUF
                # tensor_tensor runs 2x there, and keeping them off the Pool
                # engine avoids the shared DVE/Pool SBUF-port lock (which
                # stretched 512-wide adds to 0.8-2.5us) AND keeps the GpSimd
                # instruction queue free so collective enqueues fire promptly.
                # bf16 accumulator: the ~0.5% worst-case denominator drift is
                # 30x inside tolerance (AV accumulates in f32 psum regardless).
                accA = work.tile([128, 512], BF16, tag="uaccA", bufs=2, name="accA")
                # first exp writes straight into the accumulator: saves a DVE
                # copy per head (the AV matmul reads it before the i=1 add
                # overwrites -- WAR handled by the scheduler)
                pipe = [mk_u(0, out=accA)]
                if nkm > 1:
                    pipe.append(mk_u(1))
                if nkm > 2:
                    pipe.append(mk_u(2))
                emit_fill(fill[pos] if isinstance(fill, tuple) else fill)
                # previous head's finalize goes here: its Pool/DVE chain has
                # the 2 score-blocks + fill units above as PE shadow
                flush_fin()
                for i in range(nkm):
                    u, qoff, width = pipe.pop(0)
                    if i + 3 < nkm:
                        pipe.append(mk_u(i + 3))
                    nc.tensor.matmul(
                        ps_av[:, qoff:512],
                        v_sb[i][:, 128 * h : 128 * (h + 1)],
                        u[:, 0:width],
                        start=(i == 0),
                        stop=(i == nkm - 1),
                    )
                    if i > 0:
                        nc.vector.tensor_add(
                            out=accA[:, qoff:512],
                            in0=accA[:, qoff:512],
                            in1=u[:, 0:width],
                        )

                def finalize(h=h, pos=pos, ps_av=ps_av, accA=accA):
                    u_acc_b = accA
                    ps_s2 = psum.tile(
                        [128, 512], F32, tag="sum", bufs=1, name="ps_sum"
                    )
                    nc.tensor.matmul(ps_s2, ones_sb, u_acc_b, start=True, stop=True)
                    # ~18-bit reciprocal, 5x faster than reciprocal(): plenty
                    # for softmax denominators (well away from 0/inf edges)
                    recipB = work.tile(
                        [128, 512], F32, tag="recipB", bufs=2, name="recipB"
                    )
                    nc.vector.reciprocal_approx_fast(out=recipB, in_=ps_s2)
                    attn_t = work.tile(
                        [128, 512], BF16, tag="attnT", bufs=6, name="attn_t"
                    )
                    nc.vector.tensor_mul(out=attn_t, in0=ps_av, in1=recipB)
                    ci, cpos = chunk_loc(j, h)
                    # bounce rides the SWDGE (gpsimd) queue: on the sync queue
                    # it would sit behind megabytes of queued x/weight bulk and
                    # delay the gather doorbell by ~40us
                    nc.gpsimd.dma_start(
                        out=bounce[(j, ci)][128 * cpos : 128 * (cpos + 1), :],
                        in_=attn_t,
                    )
                    if cpos == len(CHUNKS[j][ci]) - 1:
                        nc.gpsimd.collective_compute(
                            "AllGather",
                            mybir.AluOpType.bypass,
                            replica_groups=[[0, 1, 2, 3], [4, 5, 6, 7]],
                            ins=[bounce[(j, ci)].opt()],
                            outs=[gath[(j, ci)].opt()],
                        )

                pending_fin.append(finalize)

        def emit_out(j, ob, ps, dve_only=False):
            # bf16 output: rel tolerance is 2e-2, bf16 rounding adds ~4e-3
            # worst-case absolute -- and the output DMA traffic halves
            o_sb = work.tile([128, 512], BF16, tag="o_sb", bufs=3, name="o_sb")
            if dve_only or ob % 2 == 0:
                nc.vector.tensor_scalar_add(
                    out=o_sb, in0=ps, scalar1=bd_sb[:, ob : ob + 1]
                )
            else:
                nc.scalar.add(o_sb, ps, bd_sb[:, ob : ob + 1])
            # sync queue: keeps the gpsimd queue exclusively for bounce
            # writes + gather doorbells (the latency-critical path)
            nc.sync.dma_start(
                out=io["outT"][128 * ob : 128 * (ob + 1), 512 * j : 512 * (j + 1)],
                in_=o_sb,
            )

        def c_units(j, halves=(0, 1), ptag="acc"):
            """dense for q-block j as a list of ~0.5us PE units.

            Two sequential ob-pair half-chains, each using only 2 psum banks
            (acc ring), so units can interleave into the ACT-bound attention
            heads without exceeding the 8-bank budget.  g_t tiles are
            re-DMA'd per half (sync queue has the headroom)."""
            hpc = 2  # j in {0,1}: uniform 2-head chunks
            units = []
            for p in halves:
                state = {}

                def load_g(idx, j=j, state=state):
                    ch, i = divmod(idx, 4 * hpc)
                    g_t = work.tile([128, 512], BF16, tag="ga", bufs=10, name="ga")
                    # sync queue: fills consume data gathered >=1 phase ago
                    # (bounces ride the gpsimd queue, so gathers complete
                    # early), hence these posts never head-of-line block
                    nc.sync.dma_start(
                        out=g_t, in_=gath[(j, ch)][128 * i : 128 * (i + 1), :]
                    )
                    state[("g", idx)] = g_t

                def unit(j=j, p=p, idx=0, state=state, load_g=load_g):
                    if idx == 0:
                        state["ps"] = [
                            psum.tile([128, 512], F32, tag=ptag, bufs=2,
                                      name=f"ps_c{j}{p}{ob}")
                            for ob in (2 * p, 2 * p + 1)
                        ]
                        # prefetch: keep the g_t DMA stream 5 units ahead of
                        # the matmuls so interleaved units never stall on DMA
                        for a in range(min(5, KB)):
                            load_g(a)
                    if idx + 5 < KB:
                        load_g(idx + 5)
                    ch, i = divmod(idx, 4 * hpc)
                    hd = 4 * (i // hpc) + hpc * ch + (i % hpc)
                    g_t = state.pop(("g", idx))
                    for k, ob in enumerate((2 * p, 2 * p + 1)):
                        nc.tensor.matmul(
                            state["ps"][k],
                            wd_sb[hd][:, 128 * ob : 128 * (ob + 1)],
                            g_t,
                            start=(idx == 0),
                            stop=(idx == KB - 1),
                        )
                    if idx == KB - 1:
                        for k, ob in enumerate((2 * p, 2 * p + 1)):
                            # fill units run inside ACT-bound attention: keep
                            # the bias-add off the ACT engine
                            emit_out(j, ob, state["ps"][k], dve_only=True)

                for idx in range(KB):
                    units.append(
                        (lambda u=unit, idx=idx: u(idx=idx))
                    )
            return units

        fill_q = []

        def emit_fill(n):
            for _ in range(n):
                if fill_q:
                    fill_q.pop(0)()

        def phase_c(j, mid=None):
            """dense for q-block j, straight single-pass form (4 psum banks:
            acc ring + score ring -- only legal when no attention phase is
            in flight).  ga loads are prefetched 4 deep so a transient DMA
            slowdown never head-of-line blocks the matmul stream."""
            ps_d = [
                psum.tile(
                    [128, 512], F32, tag=("acc" if ob < 2 else "score"),
                    bufs=(2 if ob < 2 else 3), name=f"ps_d{ob}",
                )
                for ob in range(4)
            ]
            gts = {}

            def load(ci, i):
                g_t = work.tile([128, 512], BF16, tag="ga", bufs=10, name="ga")
                nc.sync.dma_start(
                    out=g_t, in_=gath[(j, ci)][128 * i : 128 * (i + 1), :]
                )
                gts[(ci, i)] = g_t

            # prefetch stays within the current chunk: a post for a chunk
            # whose gather is still in flight may head-of-line block the
            # queue, so the next chunk's loads are only issued after mid()
            idx = 0
            for ci, ch in enumerate(CHUNKS[j]):
                n_i = 4 * len(ch)
                for i in range(min(4, n_i)):
                    load(ci, i)
                for i in range(n_i):
                    if i + 4 < n_i:
                        load(ci, i + 4)
                    hd = 4 * (i // len(ch)) + ch[i % len(ch)]
                    for ob in range(4):
                        nc.tensor.matmul(
                            ps_d[ob],
                            wd_sb[hd][:, 128 * ob : 128 * (ob + 1)],
                            gts[(ci, i)],
                            start=(idx == 0),
                            stop=(idx == KB - 1),
                        )
                    idx += 1
                    if idx == 2:
                        flush_fin()
                if mid is not None and ci == 0:
                    # gather-independent PE work inserted exactly where the
                    # tail would otherwise idle waiting for the last gather
                    mid()
            for ob in range(4):
                emit_out(j, ob, ps_d[ob])

        # ============ emission order ============
        # every consumer is emitted >=1 full phase after its producer.  All
        # projection phases run before the b(1)/b(3)/b(2) attention stretch so
        # the dense c(0)/c(1) units can fill the PE slack of those ACT-bound
        # phases (the psum 'acc' ring has no other user in that region, so
        # fill chains may stay open across phase boundaries).  b(3) before
        # b(2) so the last gathers feed the last dense phase c(2) with
        # maximum cushion.
        phase_a(0)
        phase_a(1)
        phase_b(0)
        load_wd()
        phase_a(2)
        phase_b(1)
        phase_a(3)
        fill_q.extend(c_units(0))
        phase_b(3, fill=6)
        fill_q.extend(c_units(1, halves=(0,)))
        # b(2)'s early chunk absorbs more fill; the chunk whose gather the
        # tail waits on last gets less, so its heads finish sooner
        phase_b(2, fill=(8, 8, 4, 4))
        # fire the last gather before the dense tail, then run the remaining
        # dense half + c(3) to hide the last two gathers' transfers.  The
        # last 8 units of the c(1) half ride the free 'av' psum ring and are
        # inserted INSIDE phase_c(2) between its two chunks, covering the
        # final gather's in-flight window so the PE never idles into a HAM
        # cold restart.
        flush_fin()
        emit_fill(len(fill_q))
        tailu = c_units(1, halves=(1,), ptag="av")
        for u in tailu[:8]:
            u()
        phase_c(3)
        phase_c(2, mid=lambda: [u() for u in tailu[8:]])
        flush_fin()


def _prep_inputs(x, position_ids, Wqkv, bqkv, Wdense, bdense):
    """Host-side sharding + bf16 pre-cast + weight re-layout."""
    bf16 = ml_dtypes.bfloat16
    inv_freq = 1.0 / (BASE ** (np.arange(0, ROT, 2, dtype=np.float32) / ROT))

    # causal masking for the diagonal 128-column strip is done on the PE:
    # identity (stationary) x mneg (moving) accumulates -1e9 above the
    # diagonal before exp
    kk = np.arange(128)[:, None]
    qq = np.arange(128)[None, :]
    idm = np.eye(128, dtype=np.float32).astype(bf16)
    mneg = np.where(qq < kk, np.float32(-1e9), np.float32(0)).astype(bf16)

    R = np.zeros((ROT, ROT), np.float32)
    R[np.arange(16), np.arange(16) + 16] = -1.0
    R[np.arange(16) + 16, np.arange(16)] = 1.0
    rt = np.ascontiguousarray(R.T).astype(bf16)

    in_maps = []
    for c in range(NCORES):
        b, g = divmod(c, G)
        heads = range(HPG * g, HPG * (g + 1))
        xTb = np.ascontiguousarray(x[b].T).astype(bf16)  # [HID, S]
        wqk = np.concatenate(
            [Wqkv[:, 384 * h : 384 * h + 256] for h in heads], axis=1
        )  # [HID, 1024]
        # -> [kb, p, nb*128+n]: kb-major so the startup sweep streams one
        # contiguous 256KB tile per contraction block
        wqk = np.ascontiguousarray(
            wqk.reshape(KB, 128, NBQK * 128)
        ).astype(bf16)
        w0at = np.ascontiguousarray(wqk[0][:, 0:128])
        wv = np.concatenate(
            [Wqkv[:, 384 * h + 256 : 384 * h + 384] for h in heads], axis=1
        ).reshape(KB, 128, 512).astype(bf16)
        bqk = np.concatenate(
            [bqkv[384 * h : 384 * h + 256] for h in heads]
        ).astype(np.float32)
        bv = np.concatenate(
            [bqkv[384 * h + 256 : 384 * h + 384] for h in heads]
        ).astype(np.float32)
        wd = np.ascontiguousarray(Wdense[:, 512 * g : 512 * (g + 1)]).reshape(
            KB, 128, 512
        ).astype(bf16)
        bd = np.ascontiguousarray(bdense[512 * g : 512 * (g + 1)]).astype(np.float32)
        ang = np.outer(inv_freq, position_ids[b].astype(np.float32))  # [16, S]
        cosE = np.concatenate([np.cos(ang)] * 2, axis=0)  # [32, S]
        sinE = np.concatenate([np.sin(ang)] * 2, axis=0)
        in_maps.append(
            {
                "xT": xTb,
                "xT0": np.ascontiguousarray(xTb[:, 0:512]),
                "wqk": wqk,
                "w0at": w0at,
                "wv": wv,
                "bqk": bqk,
                "bv": bv,
                "wd": wd,
                "bd": bd,
                "cosb": cosE.astype(bf16),
                "sinb": sinE.astype(bf16),
                "rt": rt,
                "idm": idm,
                "mneg": mneg,
            }
        )
    return in_maps


def _run(in_maps, trace=False):
    if "nc" not in _CACHE:
        _CACHE["nc"] = _build_nc()
    nc = _CACHE["nc"]
    res = bass_utils.run_bass_kernel_spmd(
        nc, in_maps, core_ids=list(range(NCORES)), trace=trace
    )
    return res


def kernel(x, position_ids, attention_mask, Wqkv, bqkv, Wdense, bdense,
           _trace=False, _return_results=False):
    x = np.asarray(x, dtype=np.float32)
    position_ids = np.asarray(position_ids)
    Wqkv = np.asarray(Wqkv, dtype=np.float32)
    bqkv = np.asarray(bqkv, dtype=np.float32)
    Wdense = np.asarray(Wdense, dtype=np.float32)
    bdense = np.asarray(bdense, dtype=np.float32)

    in_maps = _prep_inputs(x, position_ids, Wqkv, bqkv, Wdense, bdense)
    res = _run(in_maps, trace=_trace)

    y = np.empty((B, S, HID), dtype=np.float32)
    for c in range(NCORES):
        b, g = divmod(c, G)
        y[b, :, 512 * g : 512 * (g + 1)] = res.results[c]["outT"].T.astype(
            np.float32
        )
    if _return_results:
        return y, res
    return y



# revision 48
# speedup vs baseline: 1.0061x; 1.0061x over previous
"""Causal self-attention (dense transformer block) on 8 Trainium2 NeuronCores.

Problem (hardcoded): B=2, S=2048, HID=2048, NH=16, HS=128, ROT=32 (partial
rotary), causal additive mask, f32 I/O.

Sharding: core c = b*4 + g handles batch b and head-group g (4 heads).
 - Phase A (projections): qkT = (x @ Wqk_shard)^T computed from host-provided
   xT (x[b] transposed, pre-cast bf16) so the contraction dim lands on
   partitions.  Q,K are produced transposed ([d, s]); V natural ([s, d]).
   m-block 0 runs a kb-OUTER sweep across all 8 feature chains (8 psum
   banks) with kb-major weight streaming, so the PE chases the incoming
   xT0/wqk DMA stream instead of stalling ~13us on the first chain.
 - RoPE in-place on the first 32 dims of each head of Q,K; rotate_half is a
   32x32 sign-permutation matmul.  All rope elementwise ops on DVE: the
   GpSimd engine stream carries the collective doorbells and must stay clear.
 - Phase B (attention): S^T tiles [128 k, 512 q] on PE, exp on ACT (no max
   subtraction; scores are O(5)), causality structural: upper blocks skipped,
   diagonal-block masking is a -1e9 pattern accumulated INTO the score psum
   by an identity-stationary matmul (128 PE cycles; exp then yields exact
   zeros, nothing on DVE).  AV accumulated on PE.  Softmax denominators:
   exp'd tiles accumulated elementwise on DVE only (bf16 SBUF adds run 2x;
   Pool would contend on the shared SBUF port), first exp written straight
   into the accumulator, one ones-matmul per head for the cross-partition
   sum.
 - Per-chunk AllGather (groups of 4 = same batch) after every 2 heads.
   Bounce writes + doorbells ride the gpsimd/SWDGE queue (on the sync queue
   they'd sit behind MBs of queued x/weight bulk and delay gathers ~40us);
   all bulk (x, weights, ga reads, out stores) rides the sync/HWDGE queue.
 - Dense (attn @ Wdense) is split into ~0.5us PE units: c(0)/c(1) fill the
   PE slack inside the ACT-bound attention phases b(3)/b(2); the tail runs
   the leftover half + c(3) + c(2), with the last 8 fill units (on the free
   'av' psum ring) inserted between c(2)'s two chunks to cover the final
   gather's in-flight window (no HAM cold restart).
 - Head h's softmax finalize (sum matmul / recip / bounce DMA / gather) is
   emitted 2 score-blocks into head h+1 so its DVE chain never head-of-line
   blocks the PE.
 - Host reassembles the 8 [512, 2048] transposed bf16 output slices.
"""

import numpy as np
import ml_dtypes

import concourse.bass as bass
import concourse.bacc as bacc
import concourse.mybir as mybir
import concourse.tile as tile
from concourse import bass_utils

B, S, HID = 2, 2048, 2048
NH = 16
HS = 128
ROT = 32
BASE = 10000.0
G = 4            # head-groups (4 heads each)
HPG = NH // G    # heads per core = 4
NCORES = 8

MB = 4           # m (seq) blocks of 512
KB = 16          # contraction blocks of 128
NBQK = 2 * HPG   # qk feature blocks of 128 (q,k interleaved per head)
F32 = mybir.dt.float32
BF16 = mybir.dt.bfloat16
SCALE = 1.0 / float(np.sqrt(np.float32(HS)))

_CACHE = {}


def _build_nc():
    nc = bacc.Bacc(
        "TRN2", target_bir_lowering=False, debug=False, num_devices=NCORES
    )

    io = {
        "xT": nc.dram_tensor("xT", [HID, S], BF16, kind="ExternalInput"),
        # first m-block of xT, contiguous [HID, 512]: per-tile dram ranges
        # are sequential so the head DMAs aggregate into large packets
        "xT0": nc.dram_tensor("xT0", [HID, 512], BF16, kind="ExternalInput"),
        # kb-major: [kb, p, nb*128+n] -- one contiguous 256KB tile per kb
        "wqk": nc.dram_tensor("wqk", [KB, 128, NBQK * 128], BF16, kind="ExternalInput"),
        # contiguous duplicate of wqk[0][:, 0:128] so the very first matmul's
        # stationary is one 32KB sequential DMA on the early-starting queue
        "w0at": nc.dram_tensor("w0at", [128, 128], BF16, kind="ExternalInput"),
        "wv": nc.dram_tensor("wv", [KB, 128, 512], BF16, kind="ExternalInput"),
        "wd": nc.dram_tensor("wd", [KB, 128, 512], BF16, kind="ExternalInput"),
        "bqk": nc.dram_tensor("bqk", [NBQK * 128], F32, kind="ExternalInput"),
        "bv": nc.dram_tensor("bv", [HPG * 128], F32, kind="ExternalInput"),
        "bd": nc.dram_tensor("bd", [512], F32, kind="ExternalInput"),
        "cosb": nc.dram_tensor("cosb", [ROT, S], BF16, kind="ExternalInput"),
        "sinb": nc.dram_tensor("sinb", [ROT, S], BF16, kind="ExternalInput"),
        "rt": nc.dram_tensor("rt", [ROT, ROT], BF16, kind="ExternalInput"),
        "idm": nc.dram_tensor("idm", [128, 128], BF16, kind="ExternalInput"),
        "mneg": nc.dram_tensor("mneg", [128, 128], BF16, kind="ExternalInput"),
        "outT": nc.dram_tensor("outT", [512, S], BF16, kind="ExternalOutput"),
    }

    with tile.TileContext(nc) as tc:
        _kernel_body(tc, io)

    nc.compile()
    return nc


def _kernel_body(tc, io):
    nc = tc.nc
    from contextlib import ExitStack

    with ExitStack() as ctx:
        singles = ctx.enter_context(tc.tile_pool(name="singles", bufs=1))
        wpool = ctx.enter_context(tc.tile_pool(name="wpool", bufs=1))
        work = ctx.enter_context(tc.tile_pool(name="work", bufs=2))
        psum = ctx.enter_context(tc.tile_pool(name="psum", bufs=2, space="PSUM"))
        dram = ctx.enter_context(tc.tile_pool(name="dram", bufs=1, space="DRAM"))

        # ---- first-matmul critical path on the SWDGE queue: it spins up
        # ~6us before the HWDGE sync queue, so the first chain's stationary
        # and first moving tiles go here, ahead of the small constants ----
        w0a = wpool.tile([128, 128], BF16, tag="wqk0a", name="w0a")
        nc.gpsimd.dma_start(out=w0a, in_=io["w0at"][:, :])
        xbf0 = []
        for kb in range(4):
            xb = work.tile([128, 512], BF16, tag="xbf", bufs=28, name="xb")
            nc.gpsimd.dma_start(
                out=xb, in_=io["xT0"][128 * kb : 128 * (kb + 1), :]
            )
            xbf0.append(xb)
        # tiny constants next: the nb=0 bias-add gates the acc-psum ring
        bqk_sb = singles.tile([128, NBQK], F32, tag="bqk_sb")
        nc.gpsimd.dma_start(out=bqk_sb, in_=io["bqk"].rearrange("(n p) -> p n", p=128))
        bd_sb = singles.tile([128, 4], F32, tag="bd_sb")
        nc.gpsimd.dma_start(out=bd_sb, in_=io["bd"].rearrange("(n p) -> p n", p=128))
        bv_row = singles.tile([1, HPG * 128], F32, tag="bv_row")
        nc.gpsimd.dma_start(out=bv_row, in_=io["bv"][None, :])
        bvB = singles.tile([128, HPG * 128], F32, tag="bvB")
        nc.gpsimd.partition_broadcast(bvB, bv_row)
        rt_sb = singles.tile([ROT, ROT], BF16, tag="rt_sb")
        nc.gpsimd.dma_start(out=rt_sb, in_=io["rt"][:, :])
        cos_sb = singles.tile([ROT, S], BF16, tag="cos_sb")
        nc.gpsimd.dma_start(out=cos_sb, in_=io["cosb"][:, :])
        sin_sb = singles.tile([ROT, S], BF16, tag="sin_sb")
        nc.gpsimd.dma_start(out=sin_sb, in_=io["sinb"][:, :])
        ones_sb = singles.tile([128, 128], BF16, tag="ones_sb")
        nc.vector.memset(ones_sb, 1.0)

        # ---- sync (HWDGE) queue: wqk is kb-major ([kb, p, nb*128+n]) and
        # streamed interleaved with the xT0 tiles in the kb-outer sweep's
        # consumption order, so the first projection phase chases the DMA
        # stream instead of stalling on the full xT0 transfer
        wkb_sb = [None] * KB
        wkb0r = wpool.tile([128, (NBQK - 1) * 128], BF16, tag="wqk0b", name="w0b")
        nc.sync.dma_start(out=wkb0r, in_=io["wqk"][0][:, 128:])

        def wqk_slice(nb, kb):
            if kb == 0:
                return w0a if nb == 0 else wkb0r[:, 128 * (nb - 1) : 128 * nb]
            return wkb_sb[kb][:, 128 * nb : 128 * (nb + 1)]

        def load_wkb(kb, eng):
            w_t = wpool.tile(
                [128, NBQK * 128], BF16, tag=f"wkb{kb}", name=f"wkb{kb}"
            )
            eng.dma_start(out=w_t, in_=io["wqk"][kb])
            wkb_sb[kb] = w_t

        for kb in range(4, KB):
            xb = work.tile([128, 512], BF16, tag="xbf", bufs=28, name="xb")
            nc.sync.dma_start(
                out=xb, in_=io["xT0"][128 * kb : 128 * (kb + 1), :]
            )
            xbf0.append(xb)
            if kb - 3 < 8:
                load_wkb(kb - 3, nc.sync)
        # wkb 8-15 ride the otherwise-idle gpsimd queue so the sync stream
        # stays clear for the xT0 tail, the a(1) x tiles and wv
        for kb in range(8, KB):
            load_wkb(kb, nc.gpsimd)

        id_sb = singles.tile([128, 128], BF16, tag="id_sb")
        nc.sync.dma_start(out=id_sb, in_=io["idm"][:, :])
        mneg_sb = singles.tile([128, 128], BF16, tag="mneg_sb")
        nc.sync.dma_start(out=mneg_sb, in_=io["mneg"][:, :])

        wv_sb = []
        wd_sb = []

        def load_wv():
            for kb in range(KB):
                wv_t = wpool.tile([128, 512], BF16, tag=f"wv{kb}", name=f"wv{kb}")
                nc.sync.dma_start(out=wv_t, in_=io["wv"][kb])
                wv_sb.append(wv_t)

        def load_wd():
            for kb in range(KB):
                wd_t = wpool.tile([128, 512], BF16, tag=f"wd{kb}", name=f"wd{kb}")
                nc.sync.dma_start(out=wd_t, in_=io["wd"][kb])
                wd_sb.append(wd_t)

        # ---- DRAM bounce buffers, one per (q-block, chunk) ----
        # chunked AllGathers start as soon as their heads finish.  j=2 is the
        # last attention phase: its final two chunks are single heads so the
        # last collective (the one the tail dense phase waits on) is small
        # and ends as early as possible.
        CHUNKS = {
            0: [[0, 1], [2, 3]],
            1: [[0, 1], [2, 3]],
            3: [[0, 1], [2, 3]],
            2: [[2, 3], [0, 1]],
        }

        def chunk_loc(j, h):
            for ci, ch in enumerate(CHUNKS[j]):
                if h in ch:
                    return ci, ch.index(h)

        bounce = {}
        gath = {}
        for j in range(MB):
            for ci, ch in enumerate(CHUNKS[j]):
                bounce[(j, ci)] = dram.tile(
                    [len(ch) * 128, 512], BF16,
                    tag=f"bounce{j}_{ci}", name=f"bounce{j}_{ci}",
                )
                gath[(j, ci)] = dram.tile(
                    [G * len(ch) * 128, 512], BF16,
                    tag=f"gath{j}_{ci}", name=f"gath{j}_{ci}",
                )

        # ---- persistent qkT / V tiles ----
        qkT = {}
        for nb in range(NBQK):
            for mb in range(MB):
                qkT[(nb, mb)] = wpool.tile(
                    [128, 512], BF16, tag=f"qkT_{nb}_{mb}", name=f"qkT_{nb}_{mb}"
                )
        v_sb = []
        for km in range(KB):
            v_sb.append(
                wpool.tile([128, HPG * 128], BF16, tag=f"v_{km}", name=f"v_{km}")
            )

        # deferred per-head softmax finalizers (sum matmul / recip / bounce /
        # gather): emitted a couple of PE ops into the NEXT head so the
        # Pool+DVE denominator chain never stalls the PE
        pending_fin = []

        def flush_fin():
            while pending_fin:
                pending_fin.pop(0)()

        # ============ phase emitters ============

        def phase_a(mb):
            """projections for m-block mb + RoPE"""
            if mb == 0:
                xbf = xbf0
            else:
                xbf = []
                for kb in range(KB):
                    xb = work.tile([128, 512], BF16, tag="xbf", bufs=28, name="xb")
                    nc.sync.dma_start(
                        out=xb,
                        in_=io["xT"][
                            128 * kb : 128 * (kb + 1), 512 * mb : 512 * (mb + 1)
                        ],
                    )
                    xbf.append(xb)

            csl = cos_sb[:, 512 * mb : 512 * (mb + 1)]
            ssl = sin_sb[:, 512 * mb : 512 * (mb + 1)]

            def emit_rope(nb):
                # RoPE per feature block (a' = a*cos + rot*sin, rotate_half
                # as a 32x32 sign-permutation matmul). Emitted one block
                # late so the PE never waits on the bias-add -> rope chain.
                qk = nb % 2
                a = qkT[(nb, mb)]
                ps_r = psum.tile([ROT, 512], F32, tag="score", bufs=3, name="ps_r")
                nc.tensor.matmul(ps_r, rt_sb, a[0:ROT, :], start=True, stop=True)
                # all three elementwise ops on DVE: the GpSimd engine stream
                # must stay clear -- it carries the collective enqueues, and a
                # rope op stuck behind a DVE dependency would stall them
                tq = work.tile([ROT, 512], BF16, tag=f"ropeq{qk}", bufs=1)
                nc.vector.tensor_mul(out=tq, in0=a[0:ROT, :], in1=csl)
                ts = work.tile([ROT, 512], BF16, tag=f"ropes{qk}", bufs=1)
                nc.vector.tensor_mul(out=ts, in0=ps_r, in1=ssl)
                nc.vector.tensor_add(out=a[0:ROT, :], in0=tq, in1=ts)

            if mb == 0:
                # kb-outer sweep across all 8 feature chains (one psum bank
                # each, using the full 8-bank budget -- nothing else is in
                # flight yet): each x tile is consumed 8x the moment it lands,
                # so the PE tracks the xT0/wqk DMA stream instead of stalling
                # ~13us for the full transfer.  Bias-adds all emitted before
                # the ropes so the 'score'-ring ps_r allocs never deadlock
                # against the still-open chain banks.
                ptags = [
                    ("acc", 2), ("acc", 2), ("score", 3), ("score", 3),
                    ("score", 3), ("av", 2), ("av", 2), ("sum", 1),
                ]
                pss = [
                    psum.tile([128, 512], F32, tag=t, bufs=b, name=f"ps_sw{nb}")
                    for nb, (t, b) in enumerate(ptags)
                ]
                for kb in range(KB):
                    for nb in range(NBQK):
                        nc.tensor.matmul(
                            pss[nb],
                            wqk_slice(nb, kb),
                            xbf[kb],
                            start=(kb == 0),
                            stop=(kb == KB - 1),
                        )
                for nb in range(NBQK):
                    nc.vector.tensor_scalar_add(
                        out=qkT[(nb, mb)], in0=pss[nb],
                        scalar1=bqk_sb[:, nb : nb + 1],
                    )
                for nb in range(NBQK):
                    emit_rope(nb)
            else:
                for nb in range(NBQK):
                    ps = psum.tile([128, 512], F32, tag="acc", bufs=2, name="ps_qk")
                    for kb in range(KB):
                        nc.tensor.matmul(
                            ps,
                            wqk_slice(nb, kb),
                            xbf[kb],
                            start=(kb == 0),
                            stop=(kb == KB - 1),
                        )
                    if nb == 1:
                        flush_fin()
                    nc.vector.tensor_scalar_add(
                        out=qkT[(nb, mb)], in0=ps, scalar1=bqk_sb[:, nb : nb + 1]
                    )
                    if nb > 0:
                        emit_rope(nb - 1)
                emit_rope(NBQK - 1)

            if mb == 0:
                load_wv()
            for msub in range(4):
                km = 4 * mb + msub
                ps = psum.tile([128, HPG * 128], F32, tag="acc", bufs=2, name="ps_v")
                for kb in range(KB):
                    nc.tensor.matmul(
                        ps,
                        xbf[kb][:, 128 * msub : 128 * (msub + 1)],
                        wv_sb[kb],
                        start=(kb == 0),
                        stop=(kb == KB - 1),
                    )
                nc.vector.tensor_add(out=v_sb[km], in0=ps, in1=bvB)

        def phase_b(j, fill=0):
            """attention for q-block j (all heads); per-head finalize is
            deferred into the next head (or next phase) via pending_fin.
            `fill` dense c-units are emitted per head: the attention heads
            are ACT(exp)-bound, so the PE slack absorbs the dense matmuls"""
            nkm = 4 * j + 4
            # heads visit in chunk order so each chunk's gather fires as soon
            # as its last head finishes
            order = tuple(h for ch in CHUNKS[j] for h in ch)
            for pos, h in enumerate(order):
                qt = qkT[(2 * h, j)]

                def mk_u(i, qt=qt, out=None):
                    # diagonal blocks only need q-columns >= 128*(i-4j): compute
                    # the triangular remainder; causal masking of the first 128
                    # columns is a -1e9 accumulate on the PE (identity
                    # stationary x pattern moving, 128 cycles) so exp yields
                    # exact zeros and the DVE never touches it
                    qoff = max(0, 128 * (i - 4 * j))
                    width = 512 - qoff
                    kt = qkT[(2 * h + 1, i // 4)]
                    diag = i >= 4 * j
                    ps_s = psum.tile([128, 512], F32, tag="score", bufs=3, name="ps_s")
                    nc.tensor.matmul(
                        ps_s[:, 0:width],
                        kt[:, 128 * (i % 4) : 128 * (i % 4 + 1)],
                        qt[:, qoff:512],
                        start=True,
                        stop=not diag,
                    )
                    if diag:
                        nc.tensor.matmul(
                            ps_s[:, 0:128],
                            id_sb,
                            mneg_sb,
                            start=False,
                            stop=True,
                        )
                    u = out
                    if u is None:
                        u = work.tile([128, 512], BF16, tag="u", bufs=8, name="u")
                    nc.scalar.activation(
                        out=u[:, 0:width], in_=ps_s[:, 0:width],
                        func=mybir.ActivationFunctionType.Exp, scale=SCALE,
                    )
                    return u, qoff, width

                ps_av = psum.tile([128, 512], F32, tag="av", bufs=2, name="ps_av")
                # accumulate exp'd tiles elementwise (the PE is the global
                # bottleneck; a single ones-matmul per head does the
                # cross-partition sum).  All adds on DVE: bf16 SBUF
                # tensor_tensor runs 2x there, and keeping them off the Pool
                # engine avoids the shared DVE/Pool SBUF-port lock (which
                # stretched 512-wide adds to 0.8-2.5us) AND keeps the GpSimd
                # instruction queue free so collective enqueues fire promptly.
                # bf16 accumulator: the ~0.5% worst-case denominator drift is
                # 30x inside tolerance (AV accumulates in f32 psum regardless).
                accA = work.tile([128, 512], BF16, tag="uaccA", bufs=2, name="accA")
                # first exp writes straight into the accumulator: saves a DVE
                # copy per head (the AV matmul reads it before the i=1 add
                # overwrites -- WAR handled by the scheduler)
                pipe = [mk_u(0, out=accA)]
                if nkm > 1:
                    pipe.append(mk_u(1))
                if nkm > 2:
                    pipe.append(mk_u(2))
                emit_fill(fill[pos] if isinstance(fill, tuple) else fill)
                # previous head's finalize goes here: its Pool/DVE chain has
                # the 2 score-blocks + fill units above as PE shadow
                flush_fin()
                for i in range(nkm):
                    u, qoff, width = pipe.pop(0)
                    if i + 3 < nkm:
                        pipe.append(mk_u(i + 3))
                    nc.tensor.matmul(
                        ps_av[:, qoff:512],
                        v_sb[i][:, 128 * h : 128 * (h + 1)],
                        u[:, 0:width],
                        start=(i == 0),
                        stop=(i == nkm - 1),
                    )
                    if i > 0:
                        nc.vector.tensor_add(
                            out=accA[:, qoff:512],
                            in0=accA[:, qoff:512],
                            in1=u[:, 0:width],
                        )

                def finalize(h=h, pos=pos, ps_av=ps_av, accA=accA):
                    u_acc_b = accA
                    ps_s2 = psum.tile(
                        [128, 512], F32, tag="sum", bufs=1, name="ps_sum"
                    )
                    nc.tensor.matmul(ps_s2, ones_sb, u_acc_b, start=True, stop=True)
                    # ~18-bit reciprocal, 5x faster than reciprocal(): plenty
                    # for softmax denominators (well away from 0/inf edges)
                    recipB = work.tile(
                        [128, 512], F32, tag="recipB", bufs=2, name="recipB"
                    )
                    nc.vector.reciprocal_approx_fast(out=recipB, in_=ps_s2)
                    attn_t = work.tile(
                        [128, 512], BF16, tag="attnT", bufs=6, name="attn_t"
                    )
                    nc.vector.tensor_mul(out=attn_t, in0=ps_av, in1=recipB)
                    ci, cpos = chunk_loc(j, h)
                    # bounce rides the SWDGE (gpsimd) queue: on the sync queue
                    # it would sit behind megabytes of queued x/weight bulk and
                    # delay the gather doorbell by ~40us
                    nc.gpsimd.dma_start(
                        out=bounce[(j, ci)][128 * cpos : 128 * (cpos + 1), :],
                        in_=attn_t,
                    )
                    if cpos == len(CHUNKS[j][ci]) - 1:
                        nc.gpsimd.collective_compute(
                            "AllGather",
                            mybir.AluOpType.bypass,
                            replica_groups=[[0, 1, 2, 3], [4, 5, 6, 7]],
                            ins=[bounce[(j, ci)].opt()],
                            outs=[gath[(j, ci)].opt()],
                        )

                pending_fin.append(finalize)

        def emit_out(j, ob, ps, dve_only=False, out_eng=None):
            # bf16 output: rel tolerance is 2e-2, bf16 rounding adds ~4e-3
            # worst-case absolute -- and the output DMA traffic halves
            o_sb = work.tile([128, 512], BF16, tag="o_sb", bufs=3, name="o_sb")
            if dve_only or ob % 2 == 0:
                nc.vector.tensor_scalar_add(
                    out=o_sb, in0=ps, scalar1=bd_sb[:, ob : ob + 1]
                )
            else:
                nc.scalar.add(o_sb, ps, bd_sb[:, ob : ob + 1])
            # default sync queue (keeps gpsimd free for bounces + doorbells
            # while attention runs); tail phases pass the gpsimd queue so ga
            # reads on sync never sit behind out stores
            (out_eng or nc.sync).dma_start(
                out=io["outT"][128 * ob : 128 * (ob + 1), 512 * j : 512 * (j + 1)],
                in_=o_sb,
            )

        def c_units(j, halves=(0, 1), ptag="acc", out_eng=None):
            """dense for q-block j as a list of ~0.5us PE units.

            Two sequential ob-pair half-chains, each using only 2 psum banks
            (acc ring), so units can interleave into the ACT-bound attention
            heads without exceeding the 8-bank budget.  g_t tiles are
            re-DMA'd per half (sync queue has the headroom)."""
            hpc = 2  # j in {0,1}: uniform 2-head chunks
            units = []
            for p in halves:
                state = {}

                def load_g(idx, j=j, state=state):
                    ch, i = divmod(idx, 4 * hpc)
                    g_t = work.tile([128, 512], BF16, tag="ga", bufs=10, name="ga")
                    # sync queue: fills consume data gathered >=1 phase ago
                    # (bounces ride the gpsimd queue, so gathers complete
                    # early), hence these posts never head-of-line block
                    nc.sync.dma_start(
                        out=g_t, in_=gath[(j, ch)][128 * i : 128 * (i + 1), :]
                    )
                    state[("g", idx)] = g_t

                def unit(j=j, p=p, idx=0, state=state, load_g=load_g):
                    if idx == 0:
                        state["ps"] = [
                            psum.tile([128, 512], F32, tag=ptag, bufs=2,
                                      name=f"ps_c{j}{p}{ob}")
                            for ob in (2 * p, 2 * p + 1)
                        ]
                        # prefetch: keep the g_t DMA stream 5 units ahead of
                        # the matmuls so interleaved units never stall on DMA
                        for a in range(min(5, KB)):
                            load_g(a)
                    if idx + 5 < KB:
                        load_g(idx + 5)
                    ch, i = divmod(idx, 4 * hpc)
                    hd = 4 * (i // hpc) + hpc * ch + (i % hpc)
                    g_t = state.pop(("g", idx))
                    for k, ob in enumerate((2 * p, 2 * p + 1)):
                        nc.tensor.matmul(
                            state["ps"][k],
                            wd_sb[hd][:, 128 * ob : 128 * (ob + 1)],
                            g_t,
                            start=(idx == 0),
                            stop=(idx == KB - 1),
                        )
                    if idx == KB - 1:
                        for k, ob in enumerate((2 * p, 2 * p + 1)):
                            # fill units run inside ACT-bound attention: keep
                            # the bias-add off the ACT engine
                            emit_out(j, ob, state["ps"][k], dve_only=True,
                                     out_eng=out_eng)

                for idx in range(KB):
                    units.append(
                        (lambda u=unit, idx=idx: u(idx=idx))
                    )
            return units

        fill_q = []

        def emit_fill(n):
            for _ in range(n):
                if fill_q:
                    fill_q.pop(0)()

        def phase_c(j, mid=None):
            """dense for q-block j, straight single-pass form (4 psum banks:
            acc ring + score ring -- only legal when no attention phase is
            in flight).  ga loads are prefetched 4 deep so a transient DMA
            slowdown never head-of-line blocks the matmul stream."""
            ps_d = [
                psum.tile(
                    [128, 512], F32, tag=("acc" if ob < 2 else "score"),
                    bufs=(2 if ob < 2 else 3), name=f"ps_d{ob}",
                )
                for ob in range(4)
            ]
            gts = {}

            def load(ci, i):
                g_t = work.tile([128, 512], BF16, tag="ga", bufs=10, name="ga")
                nc.sync.dma_start(
                    out=g_t, in_=gath[(j, ci)][128 * i : 128 * (i + 1), :]
                )
                gts[(ci, i)] = g_t

            # prefetch stays within the current chunk: a post for a chunk
            # whose gather is still in flight may head-of-line block the
            # queue, so the next chunk's loads are only issued after mid()
            idx = 0
            for ci, ch in enumerate(CHUNKS[j]):
                n_i = 4 * len(ch)
                for i in range(min(4, n_i)):
                    load(ci, i)
                for i in range(n_i):
                    if i + 4 < n_i:
                        load(ci, i + 4)
                    hd = 4 * (i // len(ch)) + ch[i % len(ch)]
                    for ob in range(4):
                        nc.tensor.matmul(
                            ps_d[ob],
                            wd_sb[hd][:, 128 * ob : 128 * (ob + 1)],
                            gts[(ci, i)],
                            start=(idx == 0),
                            stop=(idx == KB - 1),
                        )
                    idx += 1
                    if idx == 2:
                        flush_fin()
                if mid is not None and ci == 0:
                    # gather-independent PE work inserted exactly where the
                    # tail would otherwise idle waiting for the last gather
                    mid()
            for ob in range(4):
                emit_out(j, ob, ps_d[ob], out_eng=nc.gpsimd)

        # ============ emission order ============
        # every consumer is emitted >=1 full phase after its producer.  All
        # projection phases run before the b(1)/b(3)/b(2) attention stretch so
        # the dense c(0)/c(1) units can fill the PE slack of those ACT-bound
        # phases (the psum 'acc' ring has no other user in that region, so
        # fill chains may stay open across phase boundaries).  b(3) before
        # b(2) so the last gathers feed the last dense phase c(2) with
        # maximum cushion.
        phase_a(0)
        phase_a(1)
        phase_b(0)
        load_wd()
        phase_a(2)
        phase_b(1)
        phase_a(3)
        fill_q.extend(c_units(0))
        phase_b(3, fill=6)
        fill_q.extend(c_units(1, halves=(0,)))
        # b(2)'s early chunk absorbs more fill; the chunk whose gather the
        # tail waits on last gets less, so its heads finish sooner
        phase_b(2, fill=(8, 8, 4, 4))
        # fire the last gather before the dense tail, then run the remaining
        # dense half + c(3) to hide the last two gathers' transfers.  The
        # last 8 units of the c(1) half ride the free 'av' psum ring and are
        # inserted INSIDE phase_c(2) between its two chunks, covering the
        # final gather's in-flight window so the PE never idles into a HAM
        # cold restart.
        flush_fin()
        emit_fill(len(fill_q))
        tailu = c_units(1, halves=(1,), ptag="av", out_eng=nc.gpsimd)
        for u in tailu[:8]:
            u()
        phase_c(3)
        phase_c(2, mid=lambda: [u() for u in tailu[8:]])
        flush_fin()


def _prep_inputs(x, position_ids, Wqkv, bqkv, Wdense, bdense):
    """Host-side sharding + bf16 pre-cast + weight re-layout."""
    bf16 = ml_dtypes.bfloat16
    inv_freq = 1.0 / (BASE ** (np.arange(0, ROT, 2, dtype=np.float32) / ROT))

    # causal masking for the diagonal 128-column strip is done on the PE:
    # identity (stationary) x mneg (moving) accumulates -1e9 above the
    # diagonal before exp
    kk = np.arange(128)[:, None]
    qq = np.arange(128)[None, :]
    idm = np.eye(128, dtype=np.float32).astype(bf16)
    mneg = np.where(qq < kk, np.float32(-1e9), np.float32(0)).astype(bf16)

    R = np.zeros((ROT, ROT), np.float32)
    R[np.arange(16), np.arange(16) + 16] = -1.0
    R[np.arange(16) + 16, np.arange(16)] = 1.0
    rt = np.ascontiguousarray(R.T).astype(bf16)

    in_maps = []
    for c in range(NCORES):
        b, g = divmod(c, G)
        heads = range(HPG * g, HPG * (g + 1))
        xTb = np.ascontiguousarray(x[b].T).astype(bf16)  # [HID, S]
        wqk = np.concatenate(
            [Wqkv[:, 384 * h : 384 * h + 256] for h in heads], axis=1
        )  # [HID, 1024]
        # -> [kb, p, nb*128+n]: kb-major so the startup sweep streams one
        # contiguous 256KB tile per contraction block
        wqk = np.ascontiguousarray(
            wqk.reshape(KB, 128, NBQK * 128)
        ).astype(bf16)
        w0at = np.ascontiguousarray(wqk[0][:, 0:128])
        wv = np.concatenate(
            [Wqkv[:, 384 * h + 256 : 384 * h + 384] for h in heads], axis=1
        ).reshape(KB, 128, 512).astype(bf16)
        bqk = np.concatenate(
            [bqkv[384 * h : 384 * h + 256] for h in heads]
        ).astype(np.float32)
        bv = np.concatenate(
            [bqkv[384 * h + 256 : 384 * h + 384] for h in heads]
        ).astype(np.float32)
        wd = np.ascontiguousarray(Wdense[:, 512 * g : 512 * (g + 1)]).reshape(
            KB, 128, 512
        ).astype(bf16)
        bd = np.ascontiguousarray(bdense[512 * g : 512 * (g + 1)]).astype(np.float32)
        ang = np.outer(inv_freq, position_ids[b].astype(np.float32))  # [16, S]
        cosE = np.concatenate([np.cos(ang)] * 2, axis=0)  # [32, S]
        sinE = np.concatenate([np.sin(ang)] * 2, axis=0)
        in_maps.append(
            {
                "xT": xTb,
                "xT0": np.ascontiguousarray(xTb[:, 0:512]),
                "wqk": wqk,
                "w0at": w0at,
                "wv": wv,
                "bqk": bqk,
                "bv": bv,
                "wd": wd,
                "bd": bd,
                "cosb": cosE.astype(bf16),
                "sinb": sinE.astype(bf16),
                "rt": rt,
                "idm": idm,
                "mneg": mneg,
            }
        )
    return in_maps


def _run(in_maps, trace=False):
    if "nc" not in _CACHE:
        _CACHE["nc"] = _build_nc()
    nc = _CACHE["nc"]
    res = bass_utils.run_bass_kernel_spmd(
        nc, in_maps, core_ids=list(range(NCORES)), trace=trace
    )
    return res


def kernel(x, position_ids, attention_mask, Wqkv, bqkv, Wdense, bdense,
           _trace=False, _return_results=False):
    x = np.asarray(x, dtype=np.float32)
    position_ids = np.asarray(position_ids)
    Wqkv = np.asarray(Wqkv, dtype=np.float32)
    bqkv = np.asarray(bqkv, dtype=np.float32)
    Wdense = np.asarray(Wdense, dtype=np.float32)
    bdense = np.asarray(bdense, dtype=np.float32)

    in_maps = _prep_inputs(x, position_ids, Wqkv, bqkv, Wdense, bdense)
    res = _run(in_maps, trace=_trace)

    y = np.empty((B, S, HID), dtype=np.float32)
    for c in range(NCORES):
        b, g = divmod(c, G)
        y[b, :, 512 * g : 512 * (g + 1)] = res.results[c]["outT"].T.astype(
            np.float32
        )
    if _return_results:
        return y, res
    return y

